# revision 5
# baseline (speedup 1.0000x reference)
"""Causal self-attention on 8 TRN2 NeuronCores.

Sharding: core c = (batch b = c//2, head-group g = c%2); each core computes its
batch's QKV projections for its 8 heads, causal attention, and a column-shard
partial of the output projection. Host sums the two partials per batch (row-
sharded fc_o) — no on-device collectives.

Layout: everything feature-major ("transposed"):
  x_t   [1024, 2048]  input.T
  Q_t/K_t [512, 2048] head-dim-major (head h = rows h*64..h*64+64)
  scores_t [kpos, q]  per 128-kpos tile -> masked (diag) -> exp (const bias
                      -30, no max pass) -> probs
  V_aug [s, 8*65]     per-head 64 V columns + ones column => PV matmul yields
                      attnV rows 0..63 and softmax sums in row 64
  normalize: DVE reciprocal of sum row, PE ones-matmul broadcast, DVE mul
  out_t [1024, 2048] = Wo_g.T-slice @ attnV_t  (host: (A+B).T + bo)

All matmuls run float32r (1 row/cycle, N>=256). f32r multiplies pre-rounded
(11-bit mantissa) operands exactly; host pre-rounds DRAM inputs, on-device
producers (ACT/DVE with f32r out dtype) round intermediates.
"""

import numpy as np

import concourse.mybir as mybir
import concourse.tile as tile
from concourse import bacc
from concourse.bass_utils import run_bass_kernel_spmd

F32 = mybir.dt.float32
F32R = mybir.dt.float32r
AF = mybir.ActivationFunctionType
ALU = mybir.AluOpType

S = 2048
HID = 1024
HG = 512          # per-core head dims (8 heads x 64)
D = 64
QB = 1024         # q block (columns) per attention sweep
NEG = -1e10
CBIAS = -30.0     # constant exp bias (replaces per-row max subtraction)


def _round11(x):
    """Round float32 array to 11 mantissa bits (f32r operand precision)."""
    b = np.ascontiguousarray(x, np.float32).view(np.uint32)
    half = np.uint32(1 << 11)
    mask = np.uint32(~np.uint32((1 << 12) - 1))
    return ((b + half) & mask).view(np.float32)


def _build():
    nc = bacc.Bacc(None, target_bir_lowering=False)

    xq = nc.dram_tensor("xq", [HID, S], F32R, kind="ExternalInput")
    xk = nc.dram_tensor("xk", [HID, S], F32R, kind="ExternalInput")
    xv = nc.dram_tensor("xv", [HID, S], F32R, kind="ExternalInput")
    wq = nc.dram_tensor("wq", [HID, HG], F32R, kind="ExternalInput")
    wk = nc.dram_tensor("wk", [HID, HG], F32R, kind="ExternalInput")
    wv = nc.dram_tensor("wv", [HID, HG], F32R, kind="ExternalInput")
    wo = nc.dram_tensor("wo", [HG, HID], F32R, kind="ExternalInput")
    bq = nc.dram_tensor("bq", [HG, 1], F32, kind="ExternalInput")
    bk = nc.dram_tensor("bk", [HG, 1], F32, kind="ExternalInput")
    bv = nc.dram_tensor("bv", [HG, 1], F32, kind="ExternalInput")
    maskt = nc.dram_tensor("mask", [128, 128], F32, kind="ExternalInput")
    out_t = nc.dram_tensor("out_t", [HID, S], F32, kind="ExternalOutput")

    with tile.TileContext(nc) as tc:
        with (
            tc.tile_pool(name="wp", bufs=10) as wp,        # w k-tiles [128,512]
            tc.tile_pool(name="wop", bufs=1) as wop,       # wo tiles [128,1024] x4
            tc.tile_pool(name="xp", bufs=10) as xp,        # x tiles [128,512]
            tc.tile_pool(name="qtp", bufs=6) as qtp,       # Q_t per-qb [128,QB]
            tc.tile_pool(name="ktp", bufs=1) as ktp,       # K_t [128,S] x4
            tc.tile_pool(name="vp", bufs=1) as vp,         # V_aug [128,520] x16
            tc.tile_pool(name="pp", bufs=3) as pp,         # probs [128,QB]
            tc.tile_pool(name="avp", bufs=4) as avp,       # attnV [128,QB]
            tc.tile_pool(name="rp", bufs=2) as rp,         # recip [65,QB]
            tc.tile_pool(name="bp", bufs=1) as bp,         # bcast [64,QB]
            tc.tile_pool(name="osp", bufs=3) as osp,       # O-proj staging [128,512]
            tc.tile_pool(name="sing", bufs=1) as sing,     # constants
            tc.tile_pool(name="psS", bufs=2, space="PSUM") as psS,   # tag ps1
            tc.tile_pool(name="psA", bufs=2, space="PSUM") as psA,   # tag ps2
        ):
            # constants / biases
            mask_sb = sing.tile([128, 128], F32, tag="mask")
            nc.sync.dma_start(out=mask_sb, in_=maskt[:, :])
            negC = sing.tile([128, 1], F32, tag="negC")
            nc.vector.memset(negC[:], CBIAS)
            ones_sb = sing.tile([65, 64], F32, tag="ones")
            nc.vector.memset(ones_sb[:], 1.0)
            bq_sb, bk_sb, bv_sb = [], [], []
            for p in range(4):
                t = sing.tile([128, 1], F32, tag=f"bq{p}", name=f"bq{p}")
                nc.sync.dma_start(out=t, in_=bq[p * 128:(p + 1) * 128, :])
                bq_sb.append(t)
                t = sing.tile([128, 1], F32, tag=f"bk{p}", name=f"bk{p}")
                nc.sync.dma_start(out=t, in_=bk[p * 128:(p + 1) * 128, :])
                bk_sb.append(t)
                t = sing.tile([128, 1], F32, tag=f"bv{p}", name=f"bv{p}")
                nc.sync.dma_start(out=t, in_=bv[p * 128:(p + 1) * 128, :])
                bv_sb.append(t)
            wo_sb = []
            for k in range(4):
                t = wop.tile([128, HID], F32R, tag=f"wo{k}", name=f"wo{k}")
                nc.sync.dma_start(out=t, in_=wo[k * 128:(k + 1) * 128, :])
                wo_sb.append(t)

            def proj_weight_tiles(wdram):
                ts = []
                for k in range(8):
                    t = wp.tile([128, HG], F32R, tag="w", name="wtile")
                    nc.sync.dma_start(out=t, in_=wdram[k * 128:(k + 1) * 128, :])
                    ts.append(t)
                return ts

            def x_tiles(xdram, n):
                ts = []
                for k in range(8):
                    t = xp.tile([128, 512], F32R, tag="x", name="xtile")
                    nc.sync.dma_start(
                        out=t, in_=xdram[k * 128:(k + 1) * 128, n * 512:(n + 1) * 512]
                    )
                    ts.append(t)
                return ts

            # ---- K projection: K_t[512, S] ----
            K_t = [ktp.tile([128, S], F32R, tag=f"kt{p}", name=f"kt{p}") for p in range(4)]
            wk_sb = proj_weight_tiles(wk)
            for n in range(4):
                xkt = x_tiles(xk, n)
                for p in range(4):
                    ps = psS.tile([128, 512], F32, tag="ps1", name="psproj")
                    for k in range(8):
                        nc.tensor.matmul(
                            ps[:],
                            lhsT=wk_sb[k][:, p * 128:(p + 1) * 128],
                            rhs=xkt[k][:],
                            start=(k == 0),
                            stop=(k == 7),
                        )
                    nc.scalar.activation(
                        K_t[p][:, n * 512:(n + 1) * 512], ps[:],
                        AF.Identity, bias=bk_sb[p][:],
                    )

            # ---- V projection: V_aug[t][128, 520] (64 v-cols + ones per head) ----
            V_aug = [vp.tile([128, 8 * 65], F32R, tag=f"va{t}", name=f"va{t}") for t in range(16)]
            wv_sb = proj_weight_tiles(wv)
            for ss in range(4):
                xvt = x_tiles(xv, ss)
                for s4 in range(4):
                    t = ss * 4 + s4
                    ps = psS.tile([128, 512], F32, tag="ps1", name="psproj")
                    for k in range(8):
                        nc.tensor.matmul(
                            ps[:],
                            lhsT=xvt[k][:, s4 * 128:(s4 + 1) * 128],
                            rhs=wv_sb[k][:],
                            start=(k == 0),
                            stop=(k == 7),
                        )
                    va3 = V_aug[t][:].rearrange("p (h c) -> p h c", h=8)
                    nc.scalar.copy(
                        va3[:, :, 0:64], ps[:].rearrange("p (h c) -> p h c", h=8)
                    )
                    nc.scalar.activation(
                        va3[:, :, 64:65],
                        ps[:].rearrange("p (h c) -> p h c", h=8)[:, :, 0:1],
                        AF.Copy, bias=1.0, scale=0.0,
                    )

            # ---- per q-block: Q proj, attention, O proj ----
            for qb in range(2):
                wq_sb = proj_weight_tiles(wq)
                Qt = [qtp.tile([128, QB], F32R, tag="qt", name=f"qt{qb}_{p}") for p in range(4)]
                for half in range(2):
                    n = qb * 2 + half
                    xqt = x_tiles(xq, n)
                    for p in range(4):
                        ps = psS.tile([128, 512], F32, tag="ps1", name="psproj")
                        for k in range(8):
                            nc.tensor.matmul(
                                ps[:],
                                lhsT=wq_sb[k][:, p * 128:(p + 1) * 128],
                                rhs=xqt[k][:],
                                start=(k == 0),
                                stop=(k == 7),
                            )
                        nc.scalar.activation(
                            Qt[p][:, half * 512:(half + 1) * 512], ps[:],
                            AF.Identity, bias=bq_sb[p][:],
                        )

                av_t = [avp.tile([128, QB], F32R, tag="attnv", name=f"av{qb}_{p}") for p in range(4)]
                for h in range(8):
                    pt, hh = h // 2, (h % 2) * 64
                    nkb = 8 * (qb + 1)
                    apv = psA.tile([65, QB], F32, tag="ps2", name="apv")
                    for kb in range(nkb):
                        cs = max(0, kb * 128 - qb * QB)
                        sc = psS.tile([128, QB], F32, tag="ps1", name="scps")
                        c = cs
                        while c < QB:
                            ce = min((c // 512 + 1) * 512, QB)
                            nc.tensor.matmul(
                                sc[:, c:ce],
                                lhsT=K_t[pt][hh:hh + 64, kb * 128:(kb + 1) * 128],
                                rhs=Qt[pt][hh:hh + 64, c:ce],
                                start=True,
                                stop=True,
                            )
                            c = ce
                        if kb * 128 >= qb * QB:  # diagonal block: causal mask
                            nc.vector.tensor_tensor(
                                out=sc[:, cs:cs + 128],
                                in0=sc[:, cs:cs + 128],
                                in1=mask_sb[:],
                                op=ALU.add,
                            )
                        pr = pp.tile([128, QB], F32R, tag="probs", name="probs")
                        nc.scalar.activation(
                            pr[:, cs:QB], sc[:, cs:QB], AF.Exp,
                            bias=negC[:], scale=1.0,
                        )
                        for half in range(2):
                            lo, hi = half * 512, half * 512 + 512
                            s0 = max(lo, cs)
                            if s0 >= hi:
                                continue
                            nc.tensor.matmul(
                                apv[:, s0:hi],
                                lhsT=V_aug[kb][:, h * 65:(h + 1) * 65],
                                rhs=pr[:, s0:hi],
                                start=(kb == 0),
                                stop=(kb == qb * 8 + 4 * half + 3),
                            )
                    # normalize: recip of sum row, broadcast via PE, multiply
                    rc = rp.tile([65, QB], F32, tag="recip", name="recip")
                    nc.vector.reciprocal(rc[64:65, :], apv[64:65, :])
                    bps = psS.tile([64, QB], F32, tag="ps1", name="bps")
                    for half in range(2):
                        lo, hi = half * 512, half * 512 + 512
                        nc.tensor.matmul(
                            bps[:, lo:hi],
                            lhsT=ones_sb[64:65, :],
                            rhs=rc[64:65, lo:hi],
                            start=True,
                            stop=True,
                        )
                    bcs = bp.tile([64, QB], F32, tag="bcast", name="bcs")
                    nc.scalar.copy(bcs[:], bps[:])
                    nc.vector.tensor_tensor(
                        out=av_t[pt][hh:hh + 64, :],
                        in0=apv[0:64, :],
                        in1=bcs[:],
                        op=ALU.mult,
                    )
                    nc.vector.tensor_scalar_add(
                        av_t[pt][hh:hh + 64, :],
                        av_t[pt][hh:hh + 64, :],
                        bv_sb[pt][hh:hh + 64, :],
                    )

                # ---- O projection partial for this q block ----
                for m in range(8):
                    for half in range(2):
                        po = psA.tile([128, 512], F32, tag="ps2", name="pso")
                        for k in range(4):
                            nc.tensor.matmul(
                                po[:],
                                lhsT=wo_sb[k][:, m * 128:(m + 1) * 128],
                                rhs=av_t[k][:, half * 512:(half + 1) * 512],
                                start=(k == 0),
                                stop=(k == 3),
                            )
                        ob = osp.tile([128, 512], F32, tag="osb", name="osb")
                        nc.scalar.copy(ob[:], po[:])
                        nc.sync.dma_start(
                            out=out_t[
                                m * 128:(m + 1) * 128,
                                qb * QB + half * 512: qb * QB + (half + 1) * 512,
                            ],
                            in_=ob[:],
                        )

    nc.finalize()
    return nc


_NC = None


def kernel(query, key, value, Wq, bq, Wk, bk, Wv, bv, Wo, bo):
    global _NC
    if _NC is None:
        _NC = _build()

    query = np.asarray(query, np.float32)
    key = np.asarray(key, np.float32)
    value = np.asarray(value, np.float32)
    Wq = np.asarray(Wq, np.float32)
    Wk = np.asarray(Wk, np.float32)
    Wv = np.asarray(Wv, np.float32)
    Wo = np.asarray(Wo, np.float32)
    bq = np.asarray(bq, np.float32)
    bk = np.asarray(bk, np.float32)
    bv = np.asarray(bv, np.float32)
    bo = np.asarray(bo, np.float32)

    mask_arr = np.where(
        np.arange(128)[None, :] >= np.arange(128)[:, None], 0.0, NEG
    ).astype(np.float32)

    xq_b = [_round11(query[b].T) for b in range(4)]
    xk_b = [_round11(key[b].T) for b in range(4)]
    xv_b = [_round11(value[b].T) for b in range(4)]
    wq_g = [_round11(Wq[g * HG:(g + 1) * HG, :].T) for g in range(2)]
    wk_g = [_round11(Wk[g * HG:(g + 1) * HG, :].T) for g in range(2)]
    wv_g = [_round11(Wv[g * HG:(g + 1) * HG, :].T) for g in range(2)]
    wo_g = [_round11(Wo[:, g * HG:(g + 1) * HG].T) for g in range(2)]

    in_maps = []
    for c in range(8):
        b, g = c // 2, c % 2
        sl = slice(g * HG, (g + 1) * HG)
        in_maps.append(
            {
                "xq": xq_b[b],
                "xk": xk_b[b],
                "xv": xv_b[b],
                "wq": wq_g[g],
                "wk": wk_g[g],
                "wv": wv_g[g],
                "wo": wo_g[g],
                "bq": np.ascontiguousarray(bq[sl].reshape(HG, 1)),
                "bk": np.ascontiguousarray(bk[sl].reshape(HG, 1)),
                "bv": np.ascontiguousarray(bv[sl].reshape(HG, 1)),
                "mask": mask_arr,
            }
        )

    res = run_bass_kernel_spmd(_NC, in_maps, core_ids=list(range(8)))

    out = np.empty((4, S, HID), np.float32)
    for b in range(4):
        acc = res.results[2 * b]["out_t"] + res.results[2 * b + 1]["out_t"]
        out[b] = acc.T + bo.reshape(1, HID)
    return out


# revision 7
# speedup vs baseline: 1.0973x; 1.0973x over previous
"""Causal self-attention on 8 TRN2 NeuronCores.

Sharding: core c = (batch b = c//2, head-group g = c%2); each core computes its
batch's QKV projections for its 8 heads, causal attention, and a column-shard
partial of the output projection. Host sums the two partials per batch (row-
sharded fc_o) — no on-device collectives.

Layout: everything feature-major ("transposed"):
  x_t   [1024, 2048]  input.T
  Q_t/K_t [512, 2048] head-dim-major (head h = rows h*64..h*64+64)
  scores_t [kpos, q]  per 128-kpos tile -> masked (diag) -> exp (const bias
                      -30, no max pass) -> probs
  V_aug [s, 8*65]     per-head 64 V columns + ones column => PV matmul yields
                      attnV rows 0..63 and softmax sums in row 64
  normalize: DVE reciprocal of sum row, PE ones-matmul broadcast, DVE mul
  out_t [1024, 2048] = Wo_g.T-slice @ attnV_t  (host: (A+B).T + bo)

All matmuls run float32r (1 row/cycle, N>=256). f32r multiplies pre-rounded
(11-bit mantissa) operands exactly; host pre-rounds DRAM inputs, on-device
producers (ACT/DVE with f32r out dtype) round intermediates.
"""

import numpy as np

import concourse.mybir as mybir
import concourse.tile as tile
from concourse import bacc
from concourse.bass_utils import run_bass_kernel_spmd

F32 = mybir.dt.float32
F32R = mybir.dt.float32r
AF = mybir.ActivationFunctionType
ALU = mybir.AluOpType

S = 2048
HID = 1024
HG = 512          # per-core head dims (8 heads x 64)
D = 64
QB = 1024         # q block (columns) per attention sweep
NEG = -1e10
CBIAS = -30.0     # constant exp bias (replaces per-row max subtraction)


def _round11(x):
    """Round float32 array to 11 mantissa bits (f32r operand precision)."""
    b = np.ascontiguousarray(x, np.float32).view(np.uint32)
    half = np.uint32(1 << 11)
    mask = np.uint32(~np.uint32((1 << 12) - 1))
    return ((b + half) & mask).view(np.float32)


def _build():
    nc = bacc.Bacc(None, target_bir_lowering=False)

    xq = nc.dram_tensor("xq", [HID, S], F32R, kind="ExternalInput")
    xk = nc.dram_tensor("xk", [HID, S], F32R, kind="ExternalInput")
    xv = nc.dram_tensor("xv", [HID, S], F32R, kind="ExternalInput")
    wq = nc.dram_tensor("wq", [HID, HG], F32R, kind="ExternalInput")
    wk = nc.dram_tensor("wk", [HID, HG], F32R, kind="ExternalInput")
    wv = nc.dram_tensor("wv", [HID, HG], F32R, kind="ExternalInput")
    wo = nc.dram_tensor("wo", [HG, HID], F32R, kind="ExternalInput")
    bq = nc.dram_tensor("bq", [HG, 1], F32, kind="ExternalInput")
    bk = nc.dram_tensor("bk", [HG, 1], F32, kind="ExternalInput")
    bv = nc.dram_tensor("bv", [HG, 1], F32, kind="ExternalInput")
    maskt = nc.dram_tensor("mask", [128, 128], F32, kind="ExternalInput")
    out_t = nc.dram_tensor("out_t", [HID, S], F32, kind="ExternalOutput")

    with tile.TileContext(nc) as tc:
        with (
            tc.tile_pool(name="wp", bufs=10) as wp,        # w k-tiles [128,512]
            tc.tile_pool(name="wop", bufs=1) as wop,       # wo tiles [128,1024] x4
            tc.tile_pool(name="xp", bufs=10) as xp,        # x tiles [128,512]
            tc.tile_pool(name="qtp", bufs=6) as qtp,       # Q_t per-qb [128,QB]
            tc.tile_pool(name="ktp", bufs=1) as ktp,       # K_t [128,S] x4
            tc.tile_pool(name="vp", bufs=1) as vp,         # V_aug [128,520] x16
            tc.tile_pool(name="pp", bufs=3) as pp,         # probs [128,QB]
            tc.tile_pool(name="avp", bufs=4) as avp,       # attnV [128,QB]
            tc.tile_pool(name="rp", bufs=2) as rp,         # recip [65,QB]
            tc.tile_pool(name="bp", bufs=1) as bp,         # bcast [64,QB]
            tc.tile_pool(name="osp", bufs=3) as osp,       # O-proj staging [128,512]
            tc.tile_pool(name="sing", bufs=1) as sing,     # constants
            tc.tile_pool(name="psS", bufs=2, space="PSUM") as psS,   # tag ps1
            tc.tile_pool(name="psA", bufs=2, space="PSUM") as psA,   # tag ps2
        ):
            # constants / biases
            mask_sb = sing.tile([128, 128], F32, tag="mask")
            nc.sync.dma_start(out=mask_sb, in_=maskt[:, :])
            negC = sing.tile([128, 1], F32, tag="negC")
            nc.vector.memset(negC[:], CBIAS)
            ones_sb = sing.tile([65, 64], F32, tag="ones")
            nc.vector.memset(ones_sb[:], 1.0)
            bq_sb, bk_sb, bv_sb = [], [], []
            for p in range(4):
                t = sing.tile([128, 1], F32, tag=f"bq{p}", name=f"bq{p}")
                nc.sync.dma_start(out=t, in_=bq[p * 128:(p + 1) * 128, :])
                bq_sb.append(t)
                t = sing.tile([128, 1], F32, tag=f"bk{p}", name=f"bk{p}")
                nc.sync.dma_start(out=t, in_=bk[p * 128:(p + 1) * 128, :])
                bk_sb.append(t)
                t = sing.tile([128, 1], F32, tag=f"bv{p}", name=f"bv{p}")
                nc.sync.dma_start(out=t, in_=bv[p * 128:(p + 1) * 128, :])
                bv_sb.append(t)
            wo_sb = []
            for k in range(4):
                t = wop.tile([128, HID], F32R, tag=f"wo{k}", name=f"wo{k}")
                nc.sync.dma_start(out=t, in_=wo[k * 128:(k + 1) * 128, :])
                wo_sb.append(t)

            def proj_weight_tiles(wdram):
                ts = []
                for k in range(8):
                    t = wp.tile([128, HG], F32R, tag="w", name="wtile")
                    nc.sync.dma_start(out=t, in_=wdram[k * 128:(k + 1) * 128, :])
                    ts.append(t)
                return ts

            def x_tiles(xdram, n):
                ts = []
                for k in range(8):
                    t = xp.tile([128, 512], F32R, tag="x", name="xtile")
                    nc.sync.dma_start(
                        out=t, in_=xdram[k * 128:(k + 1) * 128, n * 512:(n + 1) * 512]
                    )
                    ts.append(t)
                return ts

            # ---- K projection: K_t[512, S] ----
            K_t = [ktp.tile([128, S], F32R, tag=f"kt{p}", name=f"kt{p}") for p in range(4)]
            wk_sb = proj_weight_tiles(wk)
            for n in range(4):
                xkt = x_tiles(xk, n)
                for p in range(4):
                    ps = psS.tile([128, 512], F32, tag="ps1", name="psproj")
                    for k in range(8):
                        nc.tensor.matmul(
                            ps[:],
                            lhsT=wk_sb[k][:, p * 128:(p + 1) * 128],
                            rhs=xkt[k][:],
                            start=(k == 0),
                            stop=(k == 7),
                        )
                    nc.scalar.activation(
                        K_t[p][:, n * 512:(n + 1) * 512], ps[:],
                        AF.Identity, bias=bk_sb[p][:],
                    )

            # ---- V projection: V_aug[t][128, 520] (64 v-cols + ones per head) ----
            V_aug = [vp.tile([128, 8 * 65], F32R, tag=f"va{t}", name=f"va{t}") for t in range(16)]
            wv_sb = proj_weight_tiles(wv)
            for ss in range(4):
                xvt = x_tiles(xv, ss)
                for s4 in range(4):
                    t = ss * 4 + s4
                    ps = psS.tile([128, 512], F32, tag="ps1", name="psproj")
                    for k in range(8):
                        nc.tensor.matmul(
                            ps[:],
                            lhsT=xvt[k][:, s4 * 128:(s4 + 1) * 128],
                            rhs=wv_sb[k][:],
                            start=(k == 0),
                            stop=(k == 7),
                        )
                    va3 = V_aug[t][:].rearrange("p (h c) -> p h c", h=8)
                    nc.scalar.copy(
                        va3[:, :, 0:64], ps[:].rearrange("p (h c) -> p h c", h=8)
                    )
                    nc.scalar.activation(
                        va3[:, :, 64:65],
                        ps[:].rearrange("p (h c) -> p h c", h=8)[:, :, 0:1],
                        AF.Copy, bias=1.0, scale=0.0,
                    )

            # ---- per q-block: Q proj, attention, O proj ----
            for qb in range(2):
                wq_sb = proj_weight_tiles(wq)
                Qt = [qtp.tile([128, QB], F32R, tag="qt", name=f"qt{qb}_{p}") for p in range(4)]
                for half in range(2):
                    n = qb * 2 + half
                    xqt = x_tiles(xq, n)
                    for p in range(4):
                        ps = psS.tile([128, 512], F32, tag="ps1", name="psproj")
                        for k in range(8):
                            nc.tensor.matmul(
                                ps[:],
                                lhsT=wq_sb[k][:, p * 128:(p + 1) * 128],
                                rhs=xqt[k][:],
                                start=(k == 0),
                                stop=(k == 7),
                            )
                        nc.scalar.activation(
                            Qt[p][:, half * 512:(half + 1) * 512], ps[:],
                            AF.Identity, bias=bq_sb[p][:],
                        )

                av_t = [avp.tile([128, QB], F32R, tag="attnv", name=f"av{qb}_{p}") for p in range(4)]
                apvs = {}

                def normalize(h):
                    pt, hh = h // 2, (h % 2) * 64
                    apv = apvs.pop(h)
                    rc = rp.tile([65, QB], F32, tag="recip", name="recip")
                    nc.vector.reciprocal(rc[64:65, :], apv[64:65, :])
                    bps = psS.tile([64, QB], F32, tag="ps1", name="bps")
                    for half in range(2):
                        lo, hi = half * 512, half * 512 + 512
                        nc.tensor.matmul(
                            bps[:, lo:hi],
                            lhsT=ones_sb[64:65, :],
                            rhs=rc[64:65, lo:hi],
                            start=True,
                            stop=True,
                        )
                    bcs = bp.tile([64, QB], F32, tag="bcast", name="bcs")
                    nc.scalar.copy(bcs[:], bps[:])
                    nc.vector.tensor_tensor(
                        out=av_t[pt][hh:hh + 64, :],
                        in0=apv[0:64, :],
                        in1=bcs[:],
                        op=ALU.mult,
                    )
                    nc.vector.tensor_scalar_add(
                        av_t[pt][hh:hh + 64, :],
                        av_t[pt][hh:hh + 64, :],
                        bv_sb[pt][hh:hh + 64, :],
                    )

                for h in range(8):
                    pt, hh = h // 2, (h % 2) * 64
                    nkb = 8 * (qb + 1)
                    apv = psA.tile([65, QB], F32, tag="ps2", name="apv")
                    apvs[h] = apv
                    for kb in range(nkb):
                        cs = max(0, kb * 128 - qb * QB)
                        sc = psS.tile([128, QB], F32, tag="ps1", name="scps")
                        c = cs
                        while c < QB:
                            ce = min((c // 512 + 1) * 512, QB)
                            nc.tensor.matmul(
                                sc[:, c:ce],
                                lhsT=K_t[pt][hh:hh + 64, kb * 128:(kb + 1) * 128],
                                rhs=Qt[pt][hh:hh + 64, c:ce],
                                start=True,
                                stop=True,
                            )
                            c = ce
                        if kb * 128 >= qb * QB:  # diagonal block: causal mask
                            nc.vector.tensor_tensor(
                                out=sc[:, cs:cs + 128],
                                in0=sc[:, cs:cs + 128],
                                in1=mask_sb[:],
                                op=ALU.add,
                            )
                        pr = pp.tile([128, QB], F32R, tag="probs", name="probs")
                        nc.scalar.activation(
                            pr[:, cs:QB], sc[:, cs:QB], AF.Exp,
                            bias=negC[:], scale=1.0,
                        )
                        for half in range(2):
                            lo, hi = half * 512, half * 512 + 512
                            s0 = max(lo, cs)
                            if s0 >= hi:
                                continue
                            nc.tensor.matmul(
                                apv[:, s0:hi],
                                lhsT=V_aug[kb][:, h * 65:(h + 1) * 65],
                                rhs=pr[:, s0:hi],
                                start=(kb == 0),
                                stop=(kb == qb * 8 + 4 * half + 3),
                            )
                    if h > 0:
                        normalize(h - 1)
                if True:
                    normalize(7)

                # ---- O projection partial for this q block ----
                for m in range(8):
                    for half in range(2):
                        po = psA.tile([128, 512], F32, tag="ps2", name="pso")
                        for k in range(4):
                            nc.tensor.matmul(
                                po[:],
                                lhsT=wo_sb[k][:, m * 128:(m + 1) * 128],
                                rhs=av_t[k][:, half * 512:(half + 1) * 512],
                                start=(k == 0),
                                stop=(k == 3),
                            )
                        ob = osp.tile([128, 512], F32, tag="osb", name="osb")
                        nc.scalar.copy(ob[:], po[:])
                        nc.sync.dma_start(
                            out=out_t[
                                m * 128:(m + 1) * 128,
                                qb * QB + half * 512: qb * QB + (half + 1) * 512,
                            ],
                            in_=ob[:],
                        )

    nc.finalize()
    return nc


_NC = None


def kernel(query, key, value, Wq, bq, Wk, bk, Wv, bv, Wo, bo):
    global _NC
    if _NC is None:
        _NC = _build()

    query = np.asarray(query, np.float32)
    key = np.asarray(key, np.float32)
    value = np.asarray(value, np.float32)
    Wq = np.asarray(Wq, np.float32)
    Wk = np.asarray(Wk, np.float32)
    Wv = np.asarray(Wv, np.float32)
    Wo = np.asarray(Wo, np.float32)
    bq = np.asarray(bq, np.float32)
    bk = np.asarray(bk, np.float32)
    bv = np.asarray(bv, np.float32)
    bo = np.asarray(bo, np.float32)

    mask_arr = np.where(
        np.arange(128)[None, :] >= np.arange(128)[:, None], 0.0, NEG
    ).astype(np.float32)

    xq_b = [_round11(query[b].T) for b in range(4)]
    xk_b = [_round11(key[b].T) for b in range(4)]
    xv_b = [_round11(value[b].T) for b in range(4)]
    wq_g = [_round11(Wq[g * HG:(g + 1) * HG, :].T) for g in range(2)]
    wk_g = [_round11(Wk[g * HG:(g + 1) * HG, :].T) for g in range(2)]
    wv_g = [_round11(Wv[g * HG:(g + 1) * HG, :].T) for g in range(2)]
    wo_g = [_round11(Wo[:, g * HG:(g + 1) * HG].T) for g in range(2)]

    in_maps = []
    for c in range(8):
        b, g = c // 2, c % 2
        sl = slice(g * HG, (g + 1) * HG)
        in_maps.append(
            {
                "xq": xq_b[b],
                "xk": xk_b[b],
                "xv": xv_b[b],
                "wq": wq_g[g],
                "wk": wk_g[g],
                "wv": wv_g[g],
                "wo": wo_g[g],
                "bq": np.ascontiguousarray(bq[sl].reshape(HG, 1)),
                "bk": np.ascontiguousarray(bk[sl].reshape(HG, 1)),
                "bv": np.ascontiguousarray(bv[sl].reshape(HG, 1)),
                "mask": mask_arr,
            }
        )

    res = run_bass_kernel_spmd(_NC, in_maps, core_ids=list(range(8)))

    out = np.empty((4, S, HID), np.float32)
    for b in range(4):
        acc = res.results[2 * b]["out_t"] + res.results[2 * b + 1]["out_t"]
        out[b] = acc.T + bo.reshape(1, HID)
    return out
